# revision 28
# baseline (speedup 1.0000x reference)
"""CARC attention processor kernel for 8 Trainium2 NeuronCores.

Reference computation (B=1, L=4096, C=640, H=10, D=64):
    q/k/v = hidden @ Wq/Wk/Wv, split into 10 heads of 64
    k_cat = [k, 0.42*K_bg], v_cat = [v, 0.42*V_bg]   (key length 8192)
    out   = softmax(q k_cat^T / 8) v_cat, heads merged, @ Wo + bo

Sharding: queries split 512 per core; every core computes all 10 heads for
its queries (k/v projections replicated per core).  Output is a disjoint
row-slice per core; the host concatenates.

All inputs are pre-cast to bf16 and pre-laid-out on the host (partition-major
chunks; bg V pre-scaled by ALPHA with its softmax-denominator ones columns
baked in), so the device does no input casting at all.  The kernel is
software-pipelined: hidden^T streams in key chunks with pair-0 attention
starting after the first chunk; each pair p+1's k/v projections and bg loads
are generator "slices" consumed inside pair p's attention loop so the
in-order PE queue keeps the exp engine fed across pair boundaries.

Softmax skips max-subtraction (scores ~N(0,1)); denominators come from a
ones-column appended to V (65th column) in the probs@V matmul; the output
bias rides as a 65th row of Wo against a ctx ones-row.  SPLIT_EXP moves a
fraction of the exp work to the Vector engine via a Schraudolph bf16
bit-trick (i16 = round(x*A + B) reinterpreted as bf16 ~= exp(x)).
"""

import numpy as np

import concourse.bass as bass
import concourse.mybir as mybir
import concourse.tile as tile

F32 = mybir.dt.float32
BF16 = mybir.dt.bfloat16
I16 = mybir.dt.int16
AF = mybir.ActivationFunctionType
ALU = mybir.AluOpType

# Problem constants (hardcoded per contract)
B, L, C = 1, 4096, 640
H, D = 10, 64
ALPHA = 0.42
N_CORES = 8
SCALE = 1.0 / np.sqrt(D)  # 0.125

Q = L // N_CORES          # 512 queries per core
NP = H // 2               # 5 head pairs
NCC = C // 128            # 5 contraction chunks
NKT = L // 128            # 32 key tiles per source
NK2 = NKT // 2            # 16 double-tile iterations per source

# exp split: iterations with (k2 % 3 == DVE_MOD) run Schraudolph exp on DVE
SPLIT_EXP = False
DVE_MOD = 2
SCHRAUD_A = 128.0 / np.log(2.0)      # 184.664965
SCHRAUD_B = 16256.0 - 7.41           # 127*128 - sigma (min-RMS shift)


def emit(nc: bass.Bass, split_exp: bool = SPLIT_EXP):
    hTb = nc.declare_dram_parameter("hTb", [128, NCC, L], BF16, isOutput=False)
    hqb = nc.declare_dram_parameter("hqb", [128, NCC, Q], BF16, isOutput=False)
    kbgb = nc.declare_dram_parameter("kbgb", [NP, 128, L], BF16, isOutput=False)
    vbgb = nc.declare_dram_parameter(
        "vbgb", [NP, 128, NK2, 4 * (D + 1)], BF16, isOutput=False
    )
    wqb = nc.declare_dram_parameter("wqb", [128, NCC, C], BF16, isOutput=False)
    wkb = nc.declare_dram_parameter("wkb", [128, NCC, C], BF16, isOutput=False)
    wvb = nc.declare_dram_parameter("wvb", [128, NCC, C], BF16, isOutput=False)
    wobb = nc.declare_dram_parameter("wobb", [D + 1, H, C], BF16, isOutput=False)
    out = nc.declare_dram_parameter("out", [Q, C], F32, isOutput=True)

    with tile.TileContext(nc) as tc:
        with (
            tc.tile_pool(name="singles", bufs=1) as singles,
            tc.tile_pool(name="kv", bufs=2) as kv,
            tc.tile_pool(name="probs", bufs=4) as probs_pool,
            tc.tile_pool(name="fin", bufs=2) as fin_pool,
            tc.tile_pool(name="outsb", bufs=2) as outsb_pool,
            tc.tile_pool(name="ps_a", bufs=2, space="PSUM") as ps_a,
            tc.tile_pool(name="ps_sc", bufs=2, space="PSUM") as ps_sc,
            tc.tile_pool(name="ps_ctx", bufs=1, space="PSUM") as ps_ctx,
        ):
            # ---- persistent SBUF tensors (DMA'd directly, no casting) ----
            hT_bf = singles.tile([128, NCC, L], BF16, tag="hT_bf")
            hq_bf = singles.tile([128, NCC, Q], BF16, tag="hq_bf")
            wq_bf = singles.tile([128, NCC, C], BF16, tag="wq_bf")
            wk_bf = singles.tile([128, NCC, C], BF16, tag="wk_bf")
            wv_bf = singles.tile([128, NCC, C], BF16, tag="wv_bf")
            wob_bf = singles.tile([D + 1, H, C], BF16, tag="wob_bf")
            qT2_all = singles.tile([128, NP, Q], BF16, tag="qT2_all")
            ctxT_all = singles.tile([D + 1, H, Q], BF16, tag="ctxT_all")
            ones65 = singles.tile([D + 1, 128], F32, tag="ones65")
            nc.vector.memset(ones65, 1.0)
            nc.vector.memset(ctxT_all[D : D + 1, :, :], 1.0)

            outA = singles.tile([128, Q // 128, C], F32, tag="outA")

            # ---- per-pair prep generators ----
            def kproj_slice(p, t, kT2):
                ps = ps_a.tile([128, 512], F32, tag="pa", name=f"kp{p}_{t}")
                for i in range(NCC):
                    nc.tensor.matmul(
                        ps,
                        lhsT=wk_bf[:, i, 128 * p : 128 * (p + 1)],
                        rhs=hT_bf[:, i, 512 * t : 512 * (t + 1)],
                        start=(i == 0),
                        stop=(i == NCC - 1),
                    )
                nc.vector.tensor_copy(out=kT2[:, 512 * t : 512 * (t + 1)], in_=ps)

            def vproj_slice(p, g, v2t):
                # 4 key tiles (512 keys) -> v2t[:, 2g:2g+2, (j,hi)*65+c]
                ps = ps_a.tile([128, 512], F32, tag="pa", name=f"vp{p}_{g}")
                psv = ps.rearrange("p (j n) -> p j n", j=4)
                for j in range(4):
                    kt = 4 * g + j
                    for i in range(NCC):
                        nc.tensor.matmul(
                            psv[:, j, :],
                            lhsT=hT_bf[:, i, 128 * kt : 128 * (kt + 1)],
                            rhs=wv_bf[:, i, 128 * p : 128 * (p + 1)],
                            start=(i == 0),
                            stop=(i == NCC - 1),
                        )
                dst = v2t[:, 2 * g : 2 * g + 2, :].rearrange(
                    "p a (j x) -> p a j x", j=2
                )
                src = ps.rearrange("p (a j n) -> p a j n", a=2, j=2)
                for hi in range(2):
                    nc.vector.tensor_copy(
                        out=dst[:, :, :, 65 * hi : 65 * hi + D],
                        in_=src[:, :, :, D * hi : D * (hi + 1)],
                    )

            def prep_pair(p, tiles, with_h=False, skip_bg=False):
                kT2, v2t, kbg2, vbg2 = tiles
                if not skip_bg:
                    # bg loads first: straight DMAs, no staging
                    nc.sync.dma_start(out=kbg2, in_=kbgb[p])
                    nc.sync.dma_start(out=vbg2, in_=vbgb[p])
                for t in range(8):
                    if with_h:
                        nc.sync.dma_start(
                            out=hT_bf[:, :, 512 * t : 512 * (t + 1)],
                            in_=hTb[:, :, 512 * t : 512 * (t + 1)],
                        )
                    kproj_slice(p, t, kT2)
                    yield
                    vproj_slice(p, t, v2t)
                    yield

            def alloc_kv(p):
                tiles = (
                    kv.tile([128, L], BF16, tag="kT", name=f"kT{p}"),
                    kv.tile([128, NK2, 4 * (D + 1)], BF16, tag="v2", name=f"v2{p}"),
                    kv.tile([128, L], BF16, tag="kbg", name=f"kbg{p}"),
                    kv.tile([128, NK2, 4 * (D + 1)], BF16, tag="vbg", name=f"vbg{p}"),
                )
                # self-V ones columns (bg V has them baked in on the host)
                nc.vector.memset(
                    tiles[1].rearrange("p a (f c) -> p (a f) c", c=D + 1)[:, :, D:],
                    1.0,
                )
                return tiles

            def outproj_a():
                # heads 0..7 of the output projection, interleaved into
                # pair 4's attention; heads 8..9 + add finish in the tail
                for qt in range(Q // 128):
                    for n0 in range(0, C, 512):
                        nw = min(512, C - n0)
                        ps = ps_a.tile(
                            [128, 512], F32, tag="pa", name=f"opa{qt}_{n0}"
                        )
                        for h in range(8):
                            nc.tensor.matmul(
                                ps[:, 0:nw],
                                lhsT=ctxT_all[:, h, 128 * qt : 128 * (qt + 1)],
                                rhs=wob_bf[:, h, n0 : n0 + nw],
                                start=(h == 0),
                                stop=(h == 7),
                            )
                        nc.vector.tensor_copy(
                            out=outA[:, qt, n0 : n0 + nw], in_=ps[:, 0:nw]
                        )
                        yield

            # ---- prologue: hq + Wq + pair-0 bg KV first, then q projection
            cur = alloc_kv(0)
            nc.sync.dma_start(out=hq_bf, in_=hqb[:, :, :])
            nc.sync.dma_start(out=wq_bf, in_=wqb[:, :, :])
            nc.sync.dma_start(out=cur[2], in_=kbgb[0])
            nc.sync.dma_start(out=cur[3], in_=vbgb[0])
            nc.sync.dma_start(out=wk_bf, in_=wkb[:, :, :])
            for p in range(NP):
                ps = ps_a.tile([128, Q], F32, tag="pa", name=f"qps{p}")
                for i in range(NCC):
                    nc.tensor.matmul(
                        ps,
                        lhsT=wq_bf[:, i, 128 * p : 128 * (p + 1)],
                        rhs=hq_bf[:, i, :],
                        start=(i == 0),
                        stop=(i == NCC - 1),
                    )
                nc.vector.tensor_copy(out=qT2_all[:, p, :], in_=ps)
            nc.sync.dma_start(out=wv_bf, in_=wvb[:, :, :])
            nc.sync.dma_start(out=wob_bf, in_=wobb[:, :, :])

            # ---- main: per-pair attention with interleaved next-pair prep ----
            gens = []
            gens.append(prep_pair(0, cur, with_h=True, skip_bg=True))

            for p in range(NP):
                kT2, v2t, kbg2, vbg2 = cur
                if p + 1 < NP:
                    nxt = alloc_kv(p + 1)
                    gens.append(prep_pair(p + 1, nxt, with_h=False))
                else:
                    nxt = None
                    gens.append(outproj_a())

                ctx2 = ps_ctx.tile([D + 1, 2, Q], F32, tag="ctx", name=f"ctx{p}")

                # ctx matmuls run one iteration behind the scores/exp so the
                # in-order PE queue has the NEXT scores ahead of the current
                # ctx — the exp engine never waits on the ctx chain.
                def emit_ctx(rec):
                    vv_, k2_, prs_, first_, last_ = rec
                    for hi in range(2):
                        for j in range(2):
                            nc.tensor.matmul(
                                ctx2[:, hi, :],
                                lhsT=vv_[
                                    :,
                                    k2_,
                                    (D + 1) * (2 * j + hi) : (D + 1)
                                    * (2 * j + hi + 1),
                                ],
                                rhs=prs_[hi][:, j, :],
                                start=(first_ and j == 0),
                                stop=(last_ and j == 1),
                            )

                pending = None
                # pair 0 attends bg keys first: they arrive by direct DMA
                # while the self k/v projections are still streaming in
                src_order = (1, 0) if p == 0 else (0, 1)
                for si, src in enumerate(src_order):
                    kk = kT2 if src == 0 else kbg2
                    vv = v2t if src == 0 else vbg2
                    e_scale = SCALE if src == 0 else SCALE * ALPHA
                    for k2 in range(NK2):
                        pos = si * NK2 + k2
                        if p == 0:
                            budget = 1
                        else:
                            budget = 1 if pos % 2 == 0 else 0
                        while budget > 0 and gens:
                            if next(gens[0], StopIteration) is StopIteration:
                                gens.pop(0)
                            else:
                                budget -= 1
                        first = pos == 0
                        last = pos == 2 * NK2 - 1
                        scs = [
                            ps_sc.tile(
                                [128, 2, Q], F32, tag="sc",
                                name=f"sc{p}_{src}_{k2}_{hi}",
                            )
                            for hi in range(2)
                        ]
                        for j in range(2):
                            kt = 2 * k2 + j
                            for hi in range(2):
                                nc.tensor.matmul(
                                    scs[hi][:, j, :],
                                    lhsT=kk[
                                        D * hi : D * (hi + 1),
                                        128 * kt : 128 * (kt + 1),
                                    ],
                                    rhs=qT2_all[D * hi : D * (hi + 1), p, :],
                                    start=True,
                                    stop=True,
                                    tile_position=(D * hi, 0),
                                )
                        use_dve = split_exp and (k2 % 3 == DVE_MOD)
                        prs = []
                        for hi in range(2):
                            pr = probs_pool.tile(
                                [128, 2, Q], BF16, tag="pr",
                                name=f"pr{p}_{src}_{k2}_{hi}",
                            )
                            if use_dve:
                                nc.vector.tensor_scalar(
                                    pr.bitcast(I16),
                                    scs[hi],
                                    SCHRAUD_A * e_scale,
                                    SCHRAUD_B,
                                    ALU.mult,
                                    ALU.add,
                                )
                            else:
                                nc.scalar.activation(
                                    pr, scs[hi], AF.Exp, scale=e_scale
                                )
                            prs.append(pr)
                        if pending is not None:
                            emit_ctx(pending)
                        pending = (vv, k2, prs, first, last)
                emit_ctx(pending)
                # normalize: both heads' denom rows (partition 64) broadcast
                # via K=1 fp32 matmuls into one PSUM tile (rows 0-63 / 64-127),
                # one reciprocal, then per-head mul into ctxT_all
                fin = fin_pool.tile([D + 1, 2, Q], F32, tag="fin", name=f"fin{p}")
                rin = fin_pool.tile([128, Q], F32, tag="rin", name=f"rin{p}")
                for hi in range(2):
                    nc.vector.tensor_copy(
                        out=fin[D : D + 1, hi, :], in_=ctx2[D : D + 1, hi, :]
                    )
                bc = ps_a.tile([128, Q], F32, tag="pa", name=f"bc{p}")
                for hi in range(2):
                    nc.tensor.matmul(
                        bc[D * hi : D * (hi + 1), :],
                        lhsT=ones65[D : D + 1, 0:D],
                        rhs=fin[D : D + 1, hi, :],
                        start=True,
                        stop=True,
                        tile_position=(D, D * hi),
                    )
                nc.vector.reciprocal(out=rin, in_=bc)
                for hi in range(2):
                    nc.vector.tensor_mul(
                        ctxT_all[0:D, 2 * p + hi, :],
                        ctx2[0:D, hi, :],
                        rin[D * hi : D * (hi + 1), :],
                    )
                cur = nxt

            while gens:
                if next(gens[0], StopIteration) is StopIteration:
                    gens.pop(0)

            # ---- output tail: heads 8..9, add the heads-0..7 partials ----
            for qt in range(Q // 128):
                o_sb = outsb_pool.tile([128, C], F32, tag="osb", name=f"osb{qt}")
                for n0 in range(0, C, 512):
                    nw = min(512, C - n0)
                    ps = ps_a.tile([128, 512], F32, tag="pa", name=f"op{qt}_{n0}")
                    for h in range(8, H):
                        nc.tensor.matmul(
                            ps[:, 0:nw],
                            lhsT=ctxT_all[:, h, 128 * qt : 128 * (qt + 1)],
                            rhs=wob_bf[:, h, n0 : n0 + nw],
                            start=(h == 8),
                            stop=(h == H - 1),
                        )
                    nc.vector.tensor_add(
                        o_sb[:, n0 : n0 + nw],
                        outA[:, qt, n0 : n0 + nw],
                        ps[:, 0:nw],
                    )
                nc.sync.dma_start(out=out[128 * qt : 128 * (qt + 1), :], in_=o_sb)
    return nc


def split_waits(nc, limit=1):
    """This container's walrus rejects >limit sync waits per instruction;
    hoist excess waits onto standalone EventSemaphore instructions."""
    cnt = 0
    for f in nc.m.functions:
        for bb in f.blocks:
            fixed = []
            for inst in bb.instructions:
                si = inst.sync_info
                if si is not None and len(si.on_wait) > limit:
                    waits = list(si.on_wait)
                    extra, keep = waits[:-limit], waits[-limit:]
                    for w in extra:
                        cnt += 1
                        ev = mybir.InstEventSemaphore(
                            name=f"I-waitsplit-{cnt}", ins=[], outs=[]
                        )
                        ev.engine = inst.engine
                        ev.sync_info = mybir.SyncInfo(on_wait=[w], on_update=[])
                        nc.register_instruction(ev)
                        fixed.append(ev)
                    si.on_wait = keep
                fixed.append(inst)
            bb.instructions[:] = fixed
    return cnt


def build_bass(split_exp: bool = SPLIT_EXP):
    nc = bass.Bass()
    emit(nc, split_exp=split_exp)
    split_waits(nc)
    return nc


def make_in_maps(hidden_states, K_bg, V_bg, Wq, Wk, Wv, Wo, bo):
    import ml_dtypes

    bf16 = ml_dtypes.bfloat16

    def chunked(w):  # [C, X] -> [128, NCC, X] bf16 (partition-major)
        w = np.asarray(w, np.float32)
        return np.ascontiguousarray(
            w.reshape(NCC, 128, w.shape[1]).transpose(1, 0, 2)
        ).astype(bf16)

    hT = np.asarray(hidden_states, np.float32)[0].T  # [C, L]
    hTb = chunked(hT)
    # bg K: [10, L, D] -> per-pair [128(=2 heads x 64 d), L]
    kbgb = np.ascontiguousarray(
        np.asarray(K_bg, np.float32).transpose(0, 2, 1).reshape(NP, 128, L)
    ).astype(bf16)
    # bg V: pre-scaled, (j,hi)-interleaved 65-wide slots with ones columns
    vb = (ALPHA * np.asarray(V_bg, np.float32)).reshape(NP, 2, NK2, 2, 128, D)
    arr = np.ones((NP, 128, NK2, 2, 2, D + 1), np.float32)
    arr[..., :D] = vb.transpose(0, 4, 2, 3, 1, 5)  # p, q, k2, j, hi, d
    vbgb = arr.reshape(NP, 128, NK2, 4 * (D + 1)).astype(bf16)

    WoB = np.zeros((H, D + 1, C), np.float32)
    WoB[:, :D, :] = np.asarray(Wo, np.float32).reshape(H, D, C)
    WoB[0, D, :] = np.asarray(bo, np.float32)
    wobb = np.ascontiguousarray(WoB.transpose(1, 0, 2)).astype(bf16)

    common = {
        "hTb": hTb,
        "kbgb": kbgb,
        "vbgb": vbgb,
        "wqb": chunked(np.asarray(Wq, np.float32)),
        "wkb": chunked(np.asarray(Wk, np.float32)),
        "wvb": chunked(np.asarray(Wv, np.float32)),
        "wobb": wobb,
    }
    return [
        dict(common, hqb=np.ascontiguousarray(hTb[:, :, Q * c : Q * (c + 1)]))
        for c in range(N_CORES)
    ]


_NC_CACHE = {}


def kernel(hidden_states, K_bg, V_bg, Wq, Wk, Wv, Wo, bo):
    if "nc" not in _NC_CACHE:
        _NC_CACHE["nc"] = build_bass()
    nc = _NC_CACHE["nc"]
    in_maps = make_in_maps(hidden_states, K_bg, V_bg, Wq, Wk, Wv, Wo, bo)
    from concourse import bass2jax

    results = bass2jax.run_bass_via_pjrt(nc, in_maps, n_cores=N_CORES)
    out = np.concatenate([results[c]["out"] for c in range(N_CORES)], axis=0)
    return out.reshape(B, L, C)


# revision 29
# speedup vs baseline: 1.1813x; 1.1813x over previous
"""CARC attention processor kernel for 8 Trainium2 NeuronCores.

Reference computation (B=1, L=4096, C=640, H=10, D=64):
    q/k/v = hidden @ Wq/Wk/Wv, split into 10 heads of 64
    k_cat = [k, 0.42*K_bg], v_cat = [v, 0.42*V_bg]   (key length 8192)
    out   = softmax(q k_cat^T / 8) v_cat, heads merged, @ Wo + bo

Sharding: queries split 512 per core; every core computes all 10 heads for
its queries (k/v projections replicated per core).  Output is a disjoint
row-slice per core; the host concatenates.

All inputs are pre-cast to bf16 and pre-laid-out on the host (partition-major
chunks; bg V pre-scaled by ALPHA with its softmax-denominator ones columns
baked in), so the device does no input casting at all.  The kernel is
software-pipelined: hidden^T streams in key chunks with pair-0 attention
starting after the first chunk; each pair p+1's k/v projections and bg loads
are generator "slices" consumed inside pair p's attention loop so the
in-order PE queue keeps the exp engine fed across pair boundaries.

Softmax skips max-subtraction (scores ~N(0,1)); denominators come from a
ones-column appended to V (65th column) in the probs@V matmul; the output
bias rides as a 65th row of Wo against a ctx ones-row.  SPLIT_EXP moves a
fraction of the exp work to the Vector engine via a Schraudolph bf16
bit-trick (i16 = round(x*A + B) reinterpreted as bf16 ~= exp(x)).
"""

import numpy as np

import concourse.bass as bass
import concourse.mybir as mybir
import concourse.tile as tile

F32 = mybir.dt.float32
BF16 = mybir.dt.bfloat16
I16 = mybir.dt.int16
AF = mybir.ActivationFunctionType
ALU = mybir.AluOpType

# Problem constants (hardcoded per contract)
B, L, C = 1, 4096, 640
H, D = 10, 64
ALPHA = 0.42
N_CORES = 8
SCALE = 1.0 / np.sqrt(D)  # 0.125

Q = L // N_CORES          # 512 queries per core
NP = H // 2               # 5 head pairs
NCC = C // 128            # 5 contraction chunks
NKT = L // 128            # 32 key tiles per source
NK2 = NKT // 2            # 16 double-tile iterations per source

# exp split: iterations with (k2 % 3 == DVE_MOD) run Schraudolph exp on DVE
SPLIT_EXP = False
DVE_MOD = 2
SCHRAUD_A = 128.0 / np.log(2.0)      # 184.664965
SCHRAUD_B = 16256.0 - 7.41           # 127*128 - sigma (min-RMS shift)


def emit(nc: bass.Bass, split_exp: bool = SPLIT_EXP):
    hTb = nc.declare_dram_parameter("hTb", [128, NCC, L], BF16, isOutput=False)
    hqb = nc.declare_dram_parameter("hqb", [128, NCC, Q], BF16, isOutput=False)
    kbgb = nc.declare_dram_parameter("kbgb", [NP, 128, L], BF16, isOutput=False)
    vbgb = nc.declare_dram_parameter(
        "vbgb", [NP, 128, NK2, 4 * (D + 1)], BF16, isOutput=False
    )
    wqb = nc.declare_dram_parameter("wqb", [128, NCC, C], BF16, isOutput=False)
    wkb = nc.declare_dram_parameter("wkb", [128, NCC, C], BF16, isOutput=False)
    wvb = nc.declare_dram_parameter("wvb", [128, NCC, C], BF16, isOutput=False)
    wobb = nc.declare_dram_parameter("wobb", [D + 1, H, C], BF16, isOutput=False)
    out = nc.declare_dram_parameter("out", [Q, C], F32, isOutput=True)

    with tile.TileContext(nc) as tc:
        with (
            tc.tile_pool(name="singles", bufs=1) as singles,
            tc.tile_pool(name="kv", bufs=2) as kv,
            tc.tile_pool(name="probs", bufs=4) as probs_pool,
            tc.tile_pool(name="fin", bufs=2) as fin_pool,
            tc.tile_pool(name="outsb", bufs=2) as outsb_pool,
            tc.tile_pool(name="ps_a", bufs=2, space="PSUM") as ps_a,
            tc.tile_pool(name="ps_sc", bufs=2, space="PSUM") as ps_sc,
            tc.tile_pool(name="ps_ctx", bufs=1, space="PSUM") as ps_ctx,
        ):
            # ---- persistent SBUF tensors (DMA'd directly, no casting) ----
            hT_bf = singles.tile([128, NCC, L], BF16, tag="hT_bf")
            hq_bf = singles.tile([128, NCC, Q], BF16, tag="hq_bf")
            wq_bf = singles.tile([128, NCC, C], BF16, tag="wq_bf")
            wk_bf = singles.tile([128, NCC, C], BF16, tag="wk_bf")
            wv_bf = singles.tile([128, NCC, C], BF16, tag="wv_bf")
            wob_bf = singles.tile([D + 1, H, C], BF16, tag="wob_bf")
            qT2_all = singles.tile([128, NP, Q], BF16, tag="qT2_all")
            ctxT_all = singles.tile([D + 1, H, Q], BF16, tag="ctxT_all")
            ones65 = singles.tile([D + 1, 128], F32, tag="ones65")
            nc.vector.memset(ones65, 1.0)
            nc.vector.memset(ctxT_all[D : D + 1, :, :], 1.0)

            outA = singles.tile([128, Q // 128, C], F32, tag="outA")

            # ---- per-pair prep generators ----
            def kproj_slice(p, t, kT2):
                ps = ps_a.tile([128, 512], F32, tag="pa", name=f"kp{p}_{t}")
                for i in range(NCC):
                    nc.tensor.matmul(
                        ps,
                        lhsT=wk_bf[:, i, 128 * p : 128 * (p + 1)],
                        rhs=hT_bf[:, i, 512 * t : 512 * (t + 1)],
                        start=(i == 0),
                        stop=(i == NCC - 1),
                    )
                nc.vector.tensor_copy(out=kT2[:, 512 * t : 512 * (t + 1)], in_=ps)

            def vproj_slice(p, g, v2t):
                # 4 key tiles (512 keys) -> v2t[:, 2g:2g+2, (j,hi)*65+c]
                ps = ps_a.tile([128, 512], F32, tag="pa", name=f"vp{p}_{g}")
                psv = ps.rearrange("p (j n) -> p j n", j=4)
                for j in range(4):
                    kt = 4 * g + j
                    for i in range(NCC):
                        nc.tensor.matmul(
                            psv[:, j, :],
                            lhsT=hT_bf[:, i, 128 * kt : 128 * (kt + 1)],
                            rhs=wv_bf[:, i, 128 * p : 128 * (p + 1)],
                            start=(i == 0),
                            stop=(i == NCC - 1),
                        )
                dst = v2t[:, 2 * g : 2 * g + 2, :].rearrange(
                    "p a (j x) -> p a j x", j=2
                )
                src = ps.rearrange("p (a j n) -> p a j n", a=2, j=2)
                for hi in range(2):
                    nc.vector.tensor_copy(
                        out=dst[:, :, :, 65 * hi : 65 * hi + D],
                        in_=src[:, :, :, D * hi : D * (hi + 1)],
                    )

            def prep_pair(p, tiles, with_h=False, skip_bg=False):
                kT2, v2t, kbg2, vbg2 = tiles
                if not skip_bg:
                    # bg loads first: straight DMAs, no staging
                    nc.sync.dma_start(out=kbg2, in_=kbgb[p])
                    nc.sync.dma_start(out=vbg2, in_=vbgb[p])
                for t in range(8):
                    if with_h:
                        nc.sync.dma_start(
                            out=hT_bf[:, :, 512 * t : 512 * (t + 1)],
                            in_=hTb[:, :, 512 * t : 512 * (t + 1)],
                        )
                    kproj_slice(p, t, kT2)
                    yield
                    vproj_slice(p, t, v2t)
                    yield

            def alloc_kv(p):
                tiles = (
                    kv.tile([128, L], BF16, tag="kT", name=f"kT{p}"),
                    kv.tile([128, NK2, 4 * (D + 1)], BF16, tag="v2", name=f"v2{p}"),
                    kv.tile([128, L], BF16, tag="kbg", name=f"kbg{p}"),
                    kv.tile([128, NK2, 4 * (D + 1)], BF16, tag="vbg", name=f"vbg{p}"),
                )
                # self-V ones columns (bg V has them baked in on the host)
                nc.vector.memset(
                    tiles[1].rearrange("p a (f c) -> p (a f) c", c=D + 1)[:, :, D:],
                    1.0,
                )
                return tiles

            def outproj_a():
                # heads 0..7 of the output projection, interleaved into
                # pair 4's attention; heads 8..9 + add finish in the tail
                for qt in range(Q // 128):
                    for n0 in range(0, C, 512):
                        nw = min(512, C - n0)
                        ps = ps_a.tile(
                            [128, 512], F32, tag="pa", name=f"opa{qt}_{n0}"
                        )
                        for h in range(8):
                            nc.tensor.matmul(
                                ps[:, 0:nw],
                                lhsT=ctxT_all[:, h, 128 * qt : 128 * (qt + 1)],
                                rhs=wob_bf[:, h, n0 : n0 + nw],
                                start=(h == 0),
                                stop=(h == 7),
                            )
                        nc.vector.tensor_copy(
                            out=outA[:, qt, n0 : n0 + nw], in_=ps[:, 0:nw]
                        )
                        yield

            # ---- prologue: hq + Wq + pair-0 bg KV first, then q projection
            cur = alloc_kv(0)
            nc.sync.dma_start(out=hq_bf, in_=hqb[:, :, :])
            nc.sync.dma_start(out=wq_bf, in_=wqb[:, :, :])
            nc.sync.dma_start(out=cur[2], in_=kbgb[0])
            nc.sync.dma_start(out=cur[3], in_=vbgb[0])
            nc.sync.dma_start(out=wk_bf, in_=wkb[:, :, :])
            for p in range(NP):
                ps = ps_a.tile([128, Q], F32, tag="pa", name=f"qps{p}")
                for i in range(NCC):
                    nc.tensor.matmul(
                        ps,
                        lhsT=wq_bf[:, i, 128 * p : 128 * (p + 1)],
                        rhs=hq_bf[:, i, :],
                        start=(i == 0),
                        stop=(i == NCC - 1),
                    )
                nc.vector.tensor_copy(out=qT2_all[:, p, :], in_=ps)
            nc.sync.dma_start(out=wv_bf, in_=wvb[:, :, :])
            nc.sync.dma_start(out=wob_bf, in_=wobb[:, :, :])

            # ---- main: per-pair attention with interleaved next-pair prep ----
            gens = []
            gens.append(prep_pair(0, cur, with_h=True, skip_bg=True))

            for p in range(NP):
                kT2, v2t, kbg2, vbg2 = cur
                if p + 1 < NP:
                    nxt = alloc_kv(p + 1)
                    gens.append(prep_pair(p + 1, nxt, with_h=False))
                else:
                    nxt = None
                    gens.append(outproj_a())

                ctx2 = ps_ctx.tile([D + 1, 2, Q], F32, tag="ctx", name=f"ctx{p}")

                # ctx matmuls run one iteration behind the scores/exp so the
                # in-order PE queue has the NEXT scores ahead of the current
                # ctx — the exp engine never waits on the ctx chain.
                def emit_ctx(rec):
                    vv_, k2_, prs_, first_, last_ = rec
                    for hi in range(2):
                        for j in range(2):
                            nc.tensor.matmul(
                                ctx2[:, hi, :],
                                lhsT=vv_[
                                    :,
                                    k2_,
                                    (D + 1) * (2 * j + hi) : (D + 1)
                                    * (2 * j + hi + 1),
                                ],
                                rhs=prs_[hi][:, j, :],
                                start=(first_ and j == 0),
                                stop=(last_ and j == 1),
                            )

                pending = None
                # pair 0 attends bg keys first: they arrive by direct DMA
                # while the self k/v projections are still streaming in
                src_order = (1, 0) if p == 0 else (0, 1)
                for si, src in enumerate(src_order):
                    kk = kT2 if src == 0 else kbg2
                    vv = v2t if src == 0 else vbg2
                    e_scale = SCALE if src == 0 else SCALE * ALPHA
                    for k2 in range(NK2):
                        pos = si * NK2 + k2
                        if p == 0:
                            budget = 2 if pos < 8 else (1 if pos % 2 == 0 else 0)
                        else:
                            budget = 1 if pos % 2 == 0 else 0
                        while budget > 0 and gens:
                            if next(gens[0], StopIteration) is StopIteration:
                                gens.pop(0)
                            else:
                                budget -= 1
                        first = pos == 0
                        last = pos == 2 * NK2 - 1
                        scs = [
                            ps_sc.tile(
                                [128, 2, Q], F32, tag="sc",
                                name=f"sc{p}_{src}_{k2}_{hi}",
                            )
                            for hi in range(2)
                        ]
                        for j in range(2):
                            kt = 2 * k2 + j
                            for hi in range(2):
                                nc.tensor.matmul(
                                    scs[hi][:, j, :],
                                    lhsT=kk[
                                        D * hi : D * (hi + 1),
                                        128 * kt : 128 * (kt + 1),
                                    ],
                                    rhs=qT2_all[D * hi : D * (hi + 1), p, :],
                                    start=True,
                                    stop=True,
                                    tile_position=(D * hi, 0),
                                )
                        use_dve = split_exp and (k2 % 3 == DVE_MOD)
                        prs = []
                        for hi in range(2):
                            pr = probs_pool.tile(
                                [128, 2, Q], BF16, tag="pr",
                                name=f"pr{p}_{src}_{k2}_{hi}",
                            )
                            if use_dve:
                                nc.vector.tensor_scalar(
                                    pr.bitcast(I16),
                                    scs[hi],
                                    SCHRAUD_A * e_scale,
                                    SCHRAUD_B,
                                    ALU.mult,
                                    ALU.add,
                                )
                            else:
                                nc.scalar.activation(
                                    pr, scs[hi], AF.Exp, scale=e_scale
                                )
                            prs.append(pr)
                        if pending is not None:
                            emit_ctx(pending)
                        pending = (vv, k2, prs, first, last)
                emit_ctx(pending)
                # normalize: both heads' denom rows (partition 64) broadcast
                # via K=1 fp32 matmuls into one PSUM tile (rows 0-63 / 64-127),
                # one reciprocal, then per-head mul into ctxT_all
                fin = fin_pool.tile([D + 1, 2, Q], F32, tag="fin", name=f"fin{p}")
                rin = fin_pool.tile([128, Q], F32, tag="rin", name=f"rin{p}")
                for hi in range(2):
                    nc.vector.tensor_copy(
                        out=fin[D : D + 1, hi, :], in_=ctx2[D : D + 1, hi, :]
                    )
                bc = ps_a.tile([128, Q], F32, tag="pa", name=f"bc{p}")
                for hi in range(2):
                    nc.tensor.matmul(
                        bc[D * hi : D * (hi + 1), :],
                        lhsT=ones65[D : D + 1, 0:D],
                        rhs=fin[D : D + 1, hi, :],
                        start=True,
                        stop=True,
                        tile_position=(D, D * hi),
                    )
                nc.vector.reciprocal(out=rin, in_=bc)
                for hi in range(2):
                    nc.vector.tensor_mul(
                        ctxT_all[0:D, 2 * p + hi, :],
                        ctx2[0:D, hi, :],
                        rin[D * hi : D * (hi + 1), :],
                    )
                cur = nxt

            while gens:
                if next(gens[0], StopIteration) is StopIteration:
                    gens.pop(0)

            # ---- output tail: heads 8..9, add the heads-0..7 partials ----
            for qt in range(Q // 128):
                o_sb = outsb_pool.tile([128, C], F32, tag="osb", name=f"osb{qt}")
                for n0 in range(0, C, 512):
                    nw = min(512, C - n0)
                    ps = ps_a.tile([128, 512], F32, tag="pa", name=f"op{qt}_{n0}")
                    for h in range(8, H):
                        nc.tensor.matmul(
                            ps[:, 0:nw],
                            lhsT=ctxT_all[:, h, 128 * qt : 128 * (qt + 1)],
                            rhs=wob_bf[:, h, n0 : n0 + nw],
                            start=(h == 8),
                            stop=(h == H - 1),
                        )
                    nc.vector.tensor_add(
                        o_sb[:, n0 : n0 + nw],
                        outA[:, qt, n0 : n0 + nw],
                        ps[:, 0:nw],
                    )
                nc.sync.dma_start(out=out[128 * qt : 128 * (qt + 1), :], in_=o_sb)
    return nc


def split_waits(nc, limit=1):
    """This container's walrus rejects >limit sync waits per instruction;
    hoist excess waits onto standalone EventSemaphore instructions."""
    cnt = 0
    for f in nc.m.functions:
        for bb in f.blocks:
            fixed = []
            for inst in bb.instructions:
                si = inst.sync_info
                if si is not None and len(si.on_wait) > limit:
                    waits = list(si.on_wait)
                    extra, keep = waits[:-limit], waits[-limit:]
                    for w in extra:
                        cnt += 1
                        ev = mybir.InstEventSemaphore(
                            name=f"I-waitsplit-{cnt}", ins=[], outs=[]
                        )
                        ev.engine = inst.engine
                        ev.sync_info = mybir.SyncInfo(on_wait=[w], on_update=[])
                        nc.register_instruction(ev)
                        fixed.append(ev)
                    si.on_wait = keep
                fixed.append(inst)
            bb.instructions[:] = fixed
    return cnt


def build_bass(split_exp: bool = SPLIT_EXP):
    nc = bass.Bass()
    emit(nc, split_exp=split_exp)
    split_waits(nc)
    return nc


def make_in_maps(hidden_states, K_bg, V_bg, Wq, Wk, Wv, Wo, bo):
    import ml_dtypes

    bf16 = ml_dtypes.bfloat16

    def chunked(w):  # [C, X] -> [128, NCC, X] bf16 (partition-major)
        w = np.asarray(w, np.float32)
        return np.ascontiguousarray(
            w.reshape(NCC, 128, w.shape[1]).transpose(1, 0, 2)
        ).astype(bf16)

    hT = np.asarray(hidden_states, np.float32)[0].T  # [C, L]
    hTb = chunked(hT)
    # bg K: [10, L, D] -> per-pair [128(=2 heads x 64 d), L]
    kbgb = np.ascontiguousarray(
        np.asarray(K_bg, np.float32).transpose(0, 2, 1).reshape(NP, 128, L)
    ).astype(bf16)
    # bg V: pre-scaled, (j,hi)-interleaved 65-wide slots with ones columns
    vb = (ALPHA * np.asarray(V_bg, np.float32)).reshape(NP, 2, NK2, 2, 128, D)
    arr = np.ones((NP, 128, NK2, 2, 2, D + 1), np.float32)
    arr[..., :D] = vb.transpose(0, 4, 2, 3, 1, 5)  # p, q, k2, j, hi, d
    vbgb = arr.reshape(NP, 128, NK2, 4 * (D + 1)).astype(bf16)

    WoB = np.zeros((H, D + 1, C), np.float32)
    WoB[:, :D, :] = np.asarray(Wo, np.float32).reshape(H, D, C)
    WoB[0, D, :] = np.asarray(bo, np.float32)
    wobb = np.ascontiguousarray(WoB.transpose(1, 0, 2)).astype(bf16)

    common = {
        "hTb": hTb,
        "kbgb": kbgb,
        "vbgb": vbgb,
        "wqb": chunked(np.asarray(Wq, np.float32)),
        "wkb": chunked(np.asarray(Wk, np.float32)),
        "wvb": chunked(np.asarray(Wv, np.float32)),
        "wobb": wobb,
    }
    return [
        dict(common, hqb=np.ascontiguousarray(hTb[:, :, Q * c : Q * (c + 1)]))
        for c in range(N_CORES)
    ]


_NC_CACHE = {}


def kernel(hidden_states, K_bg, V_bg, Wq, Wk, Wv, Wo, bo):
    if "nc" not in _NC_CACHE:
        _NC_CACHE["nc"] = build_bass()
    nc = _NC_CACHE["nc"]
    in_maps = make_in_maps(hidden_states, K_bg, V_bg, Wq, Wk, Wv, Wo, bo)
    from concourse import bass2jax

    results = bass2jax.run_bass_via_pjrt(nc, in_maps, n_cores=N_CORES)
    out = np.concatenate([results[c]["out"] for c in range(N_CORES)], axis=0)
    return out.reshape(B, L, C)


# revision 35
# speedup vs baseline: 1.1958x; 1.0123x over previous
"""CARC attention processor kernel for 8 Trainium2 NeuronCores.

Reference computation (B=1, L=4096, C=640, H=10, D=64):
    q/k/v = hidden @ Wq/Wk/Wv, split into 10 heads of 64
    k_cat = [k, 0.42*K_bg], v_cat = [v, 0.42*V_bg]   (key length 8192)
    out   = softmax(q k_cat^T / 8) v_cat, heads merged, @ Wo + bo

Sharding: queries split 512 per core; every core computes all 10 heads for
its queries (k/v projections replicated per core).  Output is a disjoint
row-slice per core; the host concatenates.

All inputs are pre-cast to bf16 and pre-laid-out on the host (partition-major
chunks; bg V pre-scaled by ALPHA with its softmax-denominator ones columns
baked in), so the device does no input casting at all.  The kernel is
software-pipelined: hidden^T streams in key chunks with pair-0 attention
starting after the first chunk; each pair p+1's k/v projections and bg loads
are generator "slices" consumed inside pair p's attention loop so the
in-order PE queue keeps the exp engine fed across pair boundaries.

Softmax skips max-subtraction (scores ~N(0,1)); denominators come from a
ones-column appended to V (65th column) in the probs@V matmul; the output
bias rides as a 65th row of Wo against a ctx ones-row.  SPLIT_EXP moves a
fraction of the exp work to the Vector engine via a Schraudolph bf16
bit-trick (i16 = round(x*A + B) reinterpreted as bf16 ~= exp(x)).
"""

import numpy as np

import concourse.bass as bass
import concourse.mybir as mybir
import concourse.tile as tile

F32 = mybir.dt.float32
BF16 = mybir.dt.bfloat16
I16 = mybir.dt.int16
AF = mybir.ActivationFunctionType
ALU = mybir.AluOpType

# Problem constants (hardcoded per contract)
B, L, C = 1, 4096, 640
H, D = 10, 64
ALPHA = 0.42
N_CORES = 8
SCALE = 1.0 / np.sqrt(D)  # 0.125

Q = L // N_CORES          # 512 queries per core
NP = H // 2               # 5 head pairs
NCC = C // 128            # 5 contraction chunks
NKT = L // 128            # 32 key tiles per source
NK2 = NKT // 2            # 16 double-tile iterations per source

# exp split: iterations with (k2 % 3 == DVE_MOD) run Schraudolph exp on DVE
SPLIT_EXP = False
DVE_MOD = 2
SCHRAUD_A = 128.0 / np.log(2.0)      # 184.664965
SCHRAUD_B = 16256.0 - 7.41           # 127*128 - sigma (min-RMS shift)


def emit(nc: bass.Bass, split_exp: bool = SPLIT_EXP):
    hTb = nc.declare_dram_parameter("hTb", [128, NCC, L], BF16, isOutput=False)
    hqb = nc.declare_dram_parameter("hqb", [128, NCC, Q], BF16, isOutput=False)
    kbgb = nc.declare_dram_parameter("kbgb", [NP, 128, L], BF16, isOutput=False)
    vbgb = nc.declare_dram_parameter(
        "vbgb", [NP, 128, NK2, 4 * (D + 1)], BF16, isOutput=False
    )
    wqb = nc.declare_dram_parameter("wqb", [128, NCC, C], BF16, isOutput=False)
    wkb = nc.declare_dram_parameter("wkb", [128, NCC, C], BF16, isOutput=False)
    wvb = nc.declare_dram_parameter("wvb", [128, NCC, C], BF16, isOutput=False)
    wobb = nc.declare_dram_parameter("wobb", [D + 1, H, C], BF16, isOutput=False)
    out = nc.declare_dram_parameter("out", [Q, C], F32, isOutput=True)

    with tile.TileContext(nc) as tc:
        with (
            tc.tile_pool(name="singles", bufs=1) as singles,
            tc.tile_pool(name="kv", bufs=2) as kv,
            tc.tile_pool(name="probs", bufs=4) as probs_pool,
            tc.tile_pool(name="fin", bufs=2) as fin_pool,
            tc.tile_pool(name="outsb", bufs=2) as outsb_pool,
            tc.tile_pool(name="ps_a", bufs=2, space="PSUM") as ps_a,
            tc.tile_pool(name="ps_sc", bufs=2, space="PSUM") as ps_sc,
            tc.tile_pool(name="ps_ctx", bufs=1, space="PSUM") as ps_ctx,
        ):
            # ---- persistent SBUF tensors (DMA'd directly, no casting) ----
            hT_bf = singles.tile([128, NCC, L], BF16, tag="hT_bf")
            hq_bf = singles.tile([128, NCC, Q], BF16, tag="hq_bf")
            wq_bf = singles.tile([128, NCC, C], BF16, tag="wq_bf")
            wk_bf = singles.tile([128, NCC, C], BF16, tag="wk_bf")
            wv_bf = singles.tile([128, NCC, C], BF16, tag="wv_bf")
            wob_bf = singles.tile([D + 1, H, C], BF16, tag="wob_bf")
            qT2_all = singles.tile([128, NP, Q], BF16, tag="qT2_all")
            ctxT_all = singles.tile([D + 1, H, Q], BF16, tag="ctxT_all")
            ones65 = singles.tile([D + 1, 128], F32, tag="ones65")
            nc.vector.memset(ones65, 1.0)
            nc.vector.memset(ctxT_all[D : D + 1, :, :], 1.0)

            outA = singles.tile([128, Q // 128, C], F32, tag="outA")

            # ---- per-pair prep generators (fine slices: ~half a projection
            # chunk each, consumed one per attention iteration) ----
            def kproj_slice(p, t, kT2):
                ps = ps_a.tile([128, 512], F32, tag="pa", name=f"kp{p}_{t}")
                for i in range(NCC):
                    nc.tensor.matmul(
                        ps,
                        lhsT=wk_bf[:, i, 128 * p : 128 * (p + 1)],
                        rhs=hT_bf[:, i, 512 * t : 512 * (t + 1)],
                        start=(i == 0),
                        stop=(i == NCC - 1),
                    )
                nc.vector.tensor_copy(out=kT2[:, 512 * t : 512 * (t + 1)], in_=ps)

            def vproj_slice(p, g, v2t):
                # 4 key tiles (512 keys) -> v2t[:, 2g:2g+2, (j,hi)*65+c]
                ps = ps_a.tile([128, 512], F32, tag="pa", name=f"vp{p}_{g}")
                psv = ps.rearrange("p (j n) -> p j n", j=4)
                for j in range(4):
                    kt = 4 * g + j
                    for i in range(NCC):
                        nc.tensor.matmul(
                            psv[:, j, :],
                            lhsT=hT_bf[:, i, 128 * kt : 128 * (kt + 1)],
                            rhs=wv_bf[:, i, 128 * p : 128 * (p + 1)],
                            start=(i == 0),
                            stop=(i == NCC - 1),
                        )
                dst = v2t[:, 2 * g : 2 * g + 2, :].rearrange(
                    "p a (j x) -> p a j x", j=2
                )
                src = ps.rearrange("p (a j n) -> p a j n", a=2, j=2)
                for hi in range(2):
                    nc.vector.tensor_copy(
                        out=dst[:, :, :, 65 * hi : 65 * hi + D],
                        in_=src[:, :, :, D * hi : D * (hi + 1)],
                    )

            def prep_pair(p, tiles, skip_bg=False):
                kT2, v2t, kbg2, vbg2 = tiles
                if not skip_bg:
                    # bg loads first: straight DMAs, no staging
                    nc.sync.dma_start(out=kbg2, in_=kbgb[p])
                    nc.sync.dma_start(out=vbg2, in_=vbgb[p])
                for t in range(8):
                    kproj_slice(p, t, kT2)
                    yield
                    vproj_slice(p, t, v2t)
                    yield

            def alloc_kv(p):
                tiles = (
                    kv.tile([128, L], BF16, tag="kT", name=f"kT{p}"),
                    kv.tile([128, NK2, 4 * (D + 1)], BF16, tag="v2", name=f"v2{p}"),
                    kv.tile([128, L], BF16, tag="kbg", name=f"kbg{p}"),
                    kv.tile([128, NK2, 4 * (D + 1)], BF16, tag="vbg", name=f"vbg{p}"),
                )
                # self-V ones columns (bg V has them baked in on the host)
                nc.vector.memset(
                    tiles[1].rearrange("p a (f c) -> p (a f) c", c=D + 1)[:, :, D:],
                    1.0,
                )
                return tiles

            def outproj_a():
                # heads 0..7 of the output projection, interleaved into
                # pair 4's attention; heads 8..9 + add finish in the tail
                for qt in range(Q // 128):
                    for n0 in range(0, C, 512):
                        nw = min(512, C - n0)
                        ps = ps_a.tile(
                            [128, 512], F32, tag="pa", name=f"opa{qt}_{n0}"
                        )
                        for h in range(8):
                            nc.tensor.matmul(
                                ps[:, 0:nw],
                                lhsT=ctxT_all[:, h, 128 * qt : 128 * (qt + 1)],
                                rhs=wob_bf[:, h, n0 : n0 + nw],
                                start=(h == 0),
                                stop=(h == 7),
                            )
                        nc.vector.tensor_copy(
                            out=outA[:, qt, n0 : n0 + nw], in_=ps[:, 0:nw]
                        )
                        yield

            # ---- prologue DMA order: critical-path first (q proj inputs,
            # pair-0 bg chunk 0), then weights + streamed hidden chunks
            cur = alloc_kv(0)
            nc.sync.dma_start(out=hq_bf, in_=hqb[:, :, :])
            nc.sync.dma_start(out=wq_bf, in_=wqb[:, :, :])
            nc.sync.dma_start(out=cur[2][:, 0 : L // 2], in_=kbgb[0, :, 0 : L // 2])
            nc.sync.dma_start(
                out=cur[3][:, 0 : NK2 // 2, :], in_=vbgb[0, :, 0 : NK2 // 2, :]
            )
            nc.sync.dma_start(out=wk_bf, in_=wkb[:, :, :])
            nc.sync.dma_start(out=wv_bf, in_=wvb[:, :, :])
            nc.sync.dma_start(out=cur[2][:, L // 2 :], in_=kbgb[0, :, L // 2 :])
            nc.sync.dma_start(
                out=cur[3][:, NK2 // 2 :, :], in_=vbgb[0, :, NK2 // 2 :, :]
            )
            for t in range(8):
                nc.sync.dma_start(
                    out=hT_bf[:, :, 512 * t : 512 * (t + 1)],
                    in_=hTb[:, :, 512 * t : 512 * (t + 1)],
                )
            for p in range(NP):
                ps = ps_a.tile([128, Q], F32, tag="pa", name=f"qps{p}")
                for i in range(NCC):
                    nc.tensor.matmul(
                        ps,
                        lhsT=wq_bf[:, i, 128 * p : 128 * (p + 1)],
                        rhs=hq_bf[:, i, :],
                        start=(i == 0),
                        stop=(i == NCC - 1),
                    )
                nc.vector.tensor_copy(out=qT2_all[:, p, :], in_=ps)
            nc.sync.dma_start(out=wob_bf, in_=wobb[:, :, :])

            # ---- main: per-pair attention with interleaved next-pair prep ----
            gens = []
            gens.append(prep_pair(0, cur, skip_bg=True))

            for p in range(NP):
                kT2, v2t, kbg2, vbg2 = cur
                if p + 1 < NP:
                    nxt = alloc_kv(p + 1)
                    gens.append(prep_pair(p + 1, nxt))
                else:
                    nxt = None
                    gens.append(outproj_a())

                ctx2 = ps_ctx.tile([D + 1, 2, Q], F32, tag="ctx", name=f"ctx{p}")

                # ctx matmuls run one iteration behind the scores/exp so the
                # in-order PE queue has the NEXT scores ahead of the current
                # ctx — the exp engine never waits on the ctx chain.
                def emit_ctx(rec):
                    vv_, k2_, prs_, first_, last_ = rec
                    for hi in range(2):
                        for j in range(2):
                            nc.tensor.matmul(
                                ctx2[:, hi, :],
                                lhsT=vv_[
                                    :,
                                    k2_,
                                    (D + 1) * (2 * j + hi) : (D + 1)
                                    * (2 * j + hi + 1),
                                ],
                                rhs=prs_[hi][:, j, :],
                                start=(first_ and j == 0),
                                stop=(last_ and j == 1),
                            )

                pending = None
                # pair 0 attends bg keys first: they arrive by direct DMA
                # while the self k/v projections are still streaming in
                src_order = (1, 0) if p == 0 else (0, 1)
                for si, src in enumerate(src_order):
                    kk = kT2 if src == 0 else kbg2
                    vv = v2t if src == 0 else vbg2
                    e_scale = SCALE if src == 0 else SCALE * ALPHA
                    for k2 in range(NK2):
                        pos = si * NK2 + k2
                        if p == 0:
                            budget = 2 if pos < 8 else (1 if pos % 2 == 0 else 0)
                        else:
                            budget = 1 if pos % 2 == 0 else 0
                        while budget > 0 and gens:
                            if next(gens[0], StopIteration) is StopIteration:
                                gens.pop(0)
                            else:
                                budget -= 1
                        first = pos == 0
                        last = pos == 2 * NK2 - 1
                        scs = [
                            ps_sc.tile(
                                [128, 2, Q], F32, tag="sc",
                                name=f"sc{p}_{src}_{k2}_{hi}",
                            )
                            for hi in range(2)
                        ]
                        for j in range(2):
                            kt = 2 * k2 + j
                            for hi in range(2):
                                nc.tensor.matmul(
                                    scs[hi][:, j, :],
                                    lhsT=kk[
                                        D * hi : D * (hi + 1),
                                        128 * kt : 128 * (kt + 1),
                                    ],
                                    rhs=qT2_all[D * hi : D * (hi + 1), p, :],
                                    start=True,
                                    stop=True,
                                    tile_position=(D * hi, 0),
                                )
                        use_dve = split_exp and (k2 % 3 == DVE_MOD)
                        prs = []
                        for hi in range(2):
                            pr = probs_pool.tile(
                                [128, 2, Q], BF16, tag="pr",
                                name=f"pr{p}_{src}_{k2}_{hi}",
                            )
                            if use_dve:
                                nc.vector.tensor_scalar(
                                    pr.bitcast(I16),
                                    scs[hi],
                                    SCHRAUD_A * e_scale,
                                    SCHRAUD_B,
                                    ALU.mult,
                                    ALU.add,
                                )
                            else:
                                nc.scalar.activation(
                                    pr, scs[hi], AF.Exp, scale=e_scale
                                )
                            prs.append(pr)
                        if pending is not None:
                            emit_ctx(pending)
                        pending = (vv, k2, prs, first, last)
                emit_ctx(pending)
                # normalize: both heads' denom rows (partition 64) broadcast
                # via K=1 fp32 matmuls into one PSUM tile (rows 0-63 / 64-127),
                # one reciprocal, then per-head mul into ctxT_all
                fin = fin_pool.tile([D + 1, 2, Q], F32, tag="fin", name=f"fin{p}")
                rin = fin_pool.tile([128, Q], F32, tag="rin", name=f"rin{p}")
                for hi in range(2):
                    nc.vector.tensor_copy(
                        out=fin[D : D + 1, hi, :], in_=ctx2[D : D + 1, hi, :]
                    )
                bc = ps_a.tile([128, Q], F32, tag="pa", name=f"bc{p}")
                for hi in range(2):
                    nc.tensor.matmul(
                        bc[D * hi : D * (hi + 1), :],
                        lhsT=ones65[D : D + 1, 0:D],
                        rhs=fin[D : D + 1, hi, :],
                        start=True,
                        stop=True,
                        tile_position=(D, D * hi),
                    )
                nc.vector.reciprocal(out=rin, in_=bc)
                for hi in range(2):
                    nc.vector.tensor_mul(
                        ctxT_all[0:D, 2 * p + hi, :],
                        ctx2[0:D, hi, :],
                        rin[D * hi : D * (hi + 1), :],
                    )
                cur = nxt

            while gens:
                if next(gens[0], StopIteration) is StopIteration:
                    gens.pop(0)

            # ---- output tail: heads 8..9, add the heads-0..7 partials ----
            for qt in range(Q // 128):
                o_sb = outsb_pool.tile([128, C], F32, tag="osb", name=f"osb{qt}")
                for n0 in range(0, C, 512):
                    nw = min(512, C - n0)
                    ps = ps_a.tile([128, 512], F32, tag="pa", name=f"op{qt}_{n0}")
                    for h in range(8, H):
                        nc.tensor.matmul(
                            ps[:, 0:nw],
                            lhsT=ctxT_all[:, h, 128 * qt : 128 * (qt + 1)],
                            rhs=wob_bf[:, h, n0 : n0 + nw],
                            start=(h == 8),
                            stop=(h == H - 1),
                        )
                    nc.vector.tensor_add(
                        o_sb[:, n0 : n0 + nw],
                        outA[:, qt, n0 : n0 + nw],
                        ps[:, 0:nw],
                    )
                nc.sync.dma_start(out=out[128 * qt : 128 * (qt + 1), :], in_=o_sb)
    return nc


def split_waits(nc, limit=1):
    """This container's walrus rejects >limit sync waits per instruction;
    hoist excess waits onto standalone EventSemaphore instructions."""
    cnt = 0
    for f in nc.m.functions:
        for bb in f.blocks:
            fixed = []
            for inst in bb.instructions:
                si = inst.sync_info
                if si is not None and len(si.on_wait) > limit:
                    waits = list(si.on_wait)
                    extra, keep = waits[:-limit], waits[-limit:]
                    for w in extra:
                        cnt += 1
                        ev = mybir.InstEventSemaphore(
                            name=f"I-waitsplit-{cnt}", ins=[], outs=[]
                        )
                        ev.engine = inst.engine
                        ev.sync_info = mybir.SyncInfo(on_wait=[w], on_update=[])
                        nc.register_instruction(ev)
                        fixed.append(ev)
                    si.on_wait = keep
                fixed.append(inst)
            bb.instructions[:] = fixed
    return cnt


def build_bass(split_exp: bool = SPLIT_EXP):
    nc = bass.Bass()
    emit(nc, split_exp=split_exp)
    split_waits(nc)
    return nc


def make_in_maps(hidden_states, K_bg, V_bg, Wq, Wk, Wv, Wo, bo):
    import ml_dtypes

    bf16 = ml_dtypes.bfloat16

    def chunked(w):  # [C, X] -> [128, NCC, X] bf16 (partition-major)
        w = np.asarray(w, np.float32)
        return np.ascontiguousarray(
            w.reshape(NCC, 128, w.shape[1]).transpose(1, 0, 2)
        ).astype(bf16)

    hT = np.asarray(hidden_states, np.float32)[0].T  # [C, L]
    hTb = chunked(hT)
    # bg K: [10, L, D] -> per-pair [128(=2 heads x 64 d), L]
    kbgb = np.ascontiguousarray(
        np.asarray(K_bg, np.float32).transpose(0, 2, 1).reshape(NP, 128, L)
    ).astype(bf16)
    # bg V: pre-scaled, (j,hi)-interleaved 65-wide slots with ones columns
    vb = (ALPHA * np.asarray(V_bg, np.float32)).reshape(NP, 2, NK2, 2, 128, D)
    arr = np.ones((NP, 128, NK2, 2, 2, D + 1), np.float32)
    arr[..., :D] = vb.transpose(0, 4, 2, 3, 1, 5)  # p, q, k2, j, hi, d
    vbgb = arr.reshape(NP, 128, NK2, 4 * (D + 1)).astype(bf16)

    WoB = np.zeros((H, D + 1, C), np.float32)
    WoB[:, :D, :] = np.asarray(Wo, np.float32).reshape(H, D, C)
    WoB[0, D, :] = np.asarray(bo, np.float32)
    wobb = np.ascontiguousarray(WoB.transpose(1, 0, 2)).astype(bf16)

    common = {
        "hTb": hTb,
        "kbgb": kbgb,
        "vbgb": vbgb,
        "wqb": chunked(np.asarray(Wq, np.float32)),
        "wkb": chunked(np.asarray(Wk, np.float32)),
        "wvb": chunked(np.asarray(Wv, np.float32)),
        "wobb": wobb,
    }
    return [
        dict(common, hqb=np.ascontiguousarray(hTb[:, :, Q * c : Q * (c + 1)]))
        for c in range(N_CORES)
    ]


_NC_CACHE = {}


def kernel(hidden_states, K_bg, V_bg, Wq, Wk, Wv, Wo, bo):
    if "nc" not in _NC_CACHE:
        _NC_CACHE["nc"] = build_bass()
    nc = _NC_CACHE["nc"]
    in_maps = make_in_maps(hidden_states, K_bg, V_bg, Wq, Wk, Wv, Wo, bo)
    from concourse import bass2jax

    results = bass2jax.run_bass_via_pjrt(nc, in_maps, n_cores=N_CORES)
    out = np.concatenate([results[c]["out"] for c in range(N_CORES)], axis=0)
    return out.reshape(B, L, C)
